# revision 1
# baseline (speedup 1.0000x reference)
"""Trainium2 Bass kernel for y = enc_x @ weight.T + bias.

Shapes (hardcoded): enc_x [524288, 128] f32, weight [128, 128] f32,
bias [128] f32 -> y [524288, 128] f32.

Strategy: data-parallel over 8 NeuronCores (65536 rows each). Per core the
kernel streams x through SBUF in [128, 4096] tiles where partition p holds
32 contiguous batch rows (16 KiB contiguous per partition per DMA, which is
the max-bandwidth DMA pattern). The tensor engine contracts over the
partition dim, so each 128x128 block is first PE-transposed (via identity)
into PSUM, copied to SBUF, then used as the stationary operand of a matmul
against W^T (pre-transposed on host). The matmul output lands in natural
[batch, out] layout in PSUM; the bias add is fused into the PSUM->SBUF
eviction (tensor_add against a host-broadcast bias tile). Output DMA uses
the mirror of the input access pattern, so it is also fully contiguous.
"""

import numpy as np

B, IN, OUT = 524288, 128, 128
N_CORES = 8
ROWS = B // N_CORES            # 65536 rows per core
CHUNK = 4096                   # batch rows per SBUF tile
N_CHUNKS = ROWS // CHUNK       # 16
W_PER_P = CHUNK // 128         # 32 rows per partition
FREE = CHUNK                   # SBUF tile free dim (32 blocks of 128)
GROUP = 512                    # PSUM bank: 512 f32 per partition
GROUPS = FREE // GROUP         # 8 groups of 4 blocks

_CACHE: dict = {}


def _build():
    import concourse.bacc as bacc
    import concourse.mybir as mybir
    import concourse.tile as tile
    from concourse.bass import ts

    nc = bacc.Bacc(
        "TRN2",
        target_bir_lowering=False,
        debug=False,
        enable_asserts=False,
        num_devices=N_CORES,
    )

    f32 = mybir.dt.float32
    x_d = nc.dram_tensor("x", [ROWS, IN], f32, kind="ExternalInput").ap()
    wt_d = nc.dram_tensor("wt", [IN, OUT], f32, kind="ExternalInput").ap()
    b4_d = nc.dram_tensor("b4", [128, GROUP], f32, kind="ExternalInput").ap()
    id_d = nc.dram_tensor("ident", [128, 128], f32, kind="ExternalInput").ap()
    y_d = nc.dram_tensor("y", [ROWS, OUT], f32, kind="ExternalOutput").ap()

    # partition p of chunk c holds rows c*4096 + 32p .. 32p+31 (contiguous)
    x_r = x_d.rearrange("(c p w) i -> c p (w i)", p=128, w=W_PER_P)
    y_r = y_d.rearrange("(c p w) o -> c p (w o)", p=128, w=W_PER_P)

    with tile.TileContext(nc) as tc:
        with (
            tc.tile_pool(name="consts", bufs=1) as cpool,
            tc.tile_pool(name="xin", bufs=3) as xpool,
            tc.tile_pool(name="yout", bufs=3) as ypool,
            tc.tile_pool(name="xt", bufs=6) as xtpool,
            tc.tile_pool(name="psT", bufs=3, space="PSUM") as psTpool,
            tc.tile_pool(name="psY", bufs=3, space="PSUM") as psYpool,
        ):
            wt_sb = cpool.tile([128, 128], f32)
            nc.sync.dma_start(wt_sb[:], wt_d)
            b4_sb = cpool.tile([128, GROUP], f32)
            nc.sync.dma_start(b4_sb[:], b4_d)
            id_sb = cpool.tile([128, 128], f32)
            nc.sync.dma_start(id_sb[:], id_d)

            for c in range(N_CHUNKS):
                X = xpool.tile([128, FREE], f32, tag="X")
                nc.sync.dma_start(X[:], x_r[c])
                Y = ypool.tile([128, FREE], f32, tag="Y")
                for g in range(GROUPS):
                    psT = psTpool.tile([128, GROUP], f32, tag="psT")
                    for t in range(4):
                        blk = 4 * g + t
                        nc.tensor.transpose(
                            psT[:, ts(t, 128)], X[:, ts(blk, 128)], id_sb[:]
                        )
                    xT = xtpool.tile([128, GROUP], f32, tag="xT")
                    nc.vector.tensor_copy(xT[:], psT[:])
                    psY = psYpool.tile([128, GROUP], f32, tag="psY")
                    for t in range(4):
                        nc.tensor.matmul(
                            psY[:, ts(t, 128)],
                            xT[:, ts(t, 128)],
                            wt_sb[:],
                            start=True,
                            stop=True,
                        )
                    nc.vector.tensor_add(Y[:, ts(g, GROUP)], psY[:], b4_sb[:])
                nc.sync.dma_start(y_r[c], Y[:])

    nc.compile()
    return nc


def _get_nc():
    if "nc" not in _CACHE:
        _CACHE["nc"] = _build()
    return _CACHE["nc"]


def kernel(enc_x: np.ndarray, weight: np.ndarray, bias: np.ndarray) -> np.ndarray:
    from concourse.bass_utils import run_bass_kernel_spmd

    enc_x = np.ascontiguousarray(enc_x, dtype=np.float32)
    wt = np.ascontiguousarray(weight.T.astype(np.float32))          # [IN, OUT]
    b4 = np.ascontiguousarray(
        np.tile(bias.astype(np.float32)[None, :], (128, GROUP // OUT))
    )                                                               # [128, 512]
    ident = np.eye(128, dtype=np.float32)

    in_maps = [
        {
            "x": enc_x[c * ROWS : (c + 1) * ROWS],
            "wt": wt,
            "b4": b4,
            "ident": ident,
        }
        for c in range(N_CORES)
    ]
    res = run_bass_kernel_spmd(_get_nc(), in_maps, list(range(N_CORES)))
    return np.concatenate([res.results[c]["y"] for c in range(N_CORES)], axis=0)



# revision 2
# speedup vs baseline: 2.1420x; 2.1420x over previous
"""Trainium2 Bass kernel for y = enc_x @ weight.T + bias.

Shapes (hardcoded): enc_x [524288, 128] f32, weight [128, 128] f32,
bias [128] f32 -> y [524288, 128] f32.

Strategy: data-parallel over 8 NeuronCores (65536 rows each), bf16 on the
wire. The tolerance gate (rel err < 2e-2) leaves ample room for bf16 I/O:
quantizing x and W to bf16 and the output y to bf16 gives ~4e-3 max rel
error while halving HBM traffic (the problem is memory-bound).

The host uploads x pre-transposed per core (x^T [128, 65536] bf16, feature
dim on partitions), so the device needs no on-chip transposes at all: the
tensor engine computes y^T = (W^T)^T-stationary @ x^T directly with N=512
streaming matmuls (W^T [128i, 128o] is the stationary operand, loaded from
SBUF each MM; x^T streams). PSUM fp32 accumulation; the vector engine adds
bias (per-partition scalar) while evicting PSUM -> SBUF with a bf16 cast.
The host transposes y^T back and upcasts to f32.

Input DMAs ride the SP HWDGE ring (nc.sync), output DMAs the ACT ring
(nc.scalar) so a store blocked on compute never head-of-line-blocks the
next prefetch.
"""

import numpy as np

B, IN, OUT = 524288, 128, 128
N_CORES = 8
ROWS = B // N_CORES            # 65536 rows per core
CHUNK = 4096                   # batch columns per SBUF tile (1 MiB bf16 DMA)
N_CHUNKS = ROWS // CHUNK       # 16
GROUP = 512                    # matmul N / one PSUM bank of f32
GROUPS = CHUNK // GROUP        # 8

_CACHE: dict = {}


def _build():
    import concourse.bacc as bacc
    import concourse.mybir as mybir
    import concourse.tile as tile
    from concourse.bass import ts

    nc = bacc.Bacc(
        "TRN2",
        target_bir_lowering=False,
        debug=False,
        enable_asserts=False,
        num_devices=N_CORES,
    )

    f32 = mybir.dt.float32
    bf16 = mybir.dt.bfloat16
    xt_d = nc.dram_tensor("xt", [IN, ROWS], bf16, kind="ExternalInput").ap()
    wt_d = nc.dram_tensor("wt", [IN, OUT], bf16, kind="ExternalInput").ap()
    bc_d = nc.dram_tensor("bc", [OUT, 1], f32, kind="ExternalInput").ap()
    yt_d = nc.dram_tensor("yt", [OUT, ROWS], bf16, kind="ExternalOutput").ap()

    with tile.TileContext(nc) as tc:
        with (
            tc.tile_pool(name="consts", bufs=1) as cpool,
            tc.tile_pool(name="xin", bufs=4) as xpool,
            tc.tile_pool(name="yout", bufs=4) as ypool,
            tc.tile_pool(name="ps", bufs=6, space="PSUM") as pspool,
        ):
            wt_sb = cpool.tile([IN, OUT], bf16)
            nc.sync.dma_start(wt_sb[:], wt_d)
            b_sb = cpool.tile([OUT, 1], f32)
            nc.sync.dma_start(b_sb[:], bc_d)

            for c in range(N_CHUNKS):
                X = xpool.tile([IN, CHUNK], bf16, tag="X")
                nc.sync.dma_start(X[:], xt_d[:, c * CHUNK : (c + 1) * CHUNK])
                YT = ypool.tile([OUT, CHUNK], bf16, tag="YT")
                for g in range(GROUPS):
                    ps = pspool.tile([OUT, GROUP], f32, tag="ps")
                    nc.tensor.matmul(
                        ps[:],
                        wt_sb[:],
                        X[:, ts(g, GROUP)],
                        start=True,
                        stop=True,
                    )
                    nc.vector.tensor_scalar_add(YT[:, ts(g, GROUP)], ps[:], b_sb[:])
                nc.scalar.dma_start(yt_d[:, c * CHUNK : (c + 1) * CHUNK], YT[:])

    nc.compile()
    return nc


def _get_nc():
    if "nc" not in _CACHE:
        _CACHE["nc"] = _build()
    return _CACHE["nc"]


def prep_in_maps(enc_x: np.ndarray, weight: np.ndarray, bias: np.ndarray):
    import ml_dtypes

    bf16 = ml_dtypes.bfloat16
    wt = np.ascontiguousarray(weight.astype(np.float32).T.astype(bf16))  # [IN, OUT]
    bc = np.ascontiguousarray(bias.astype(np.float32).reshape(OUT, 1))
    xb = np.asarray(enc_x, dtype=np.float32).astype(bf16)                # [B, IN]
    return [
        {
            "xt": np.ascontiguousarray(xb[c * ROWS : (c + 1) * ROWS].T),
            "wt": wt,
            "bc": bc,
        }
        for c in range(N_CORES)
    ]


def gather_output(results) -> np.ndarray:
    out = np.empty((B, OUT), dtype=np.float32)
    for c in range(N_CORES):
        yt = np.asarray(results[c]["yt"])                                # [OUT, ROWS] bf16
        out[c * ROWS : (c + 1) * ROWS] = yt.T.astype(np.float32)
    return out


def kernel(enc_x: np.ndarray, weight: np.ndarray, bias: np.ndarray) -> np.ndarray:
    from concourse.bass_utils import run_bass_kernel_spmd

    in_maps = prep_in_maps(enc_x, weight, bias)
    res = run_bass_kernel_spmd(_get_nc(), in_maps, list(range(N_CORES)))
    return gather_output(res.results)
